# revision 24
# baseline (speedup 1.0000x reference)
"""Expert-parallel MoE SwiGLU FFN kernel for 8 Trainium2 NeuronCores.

Problem: T=4096 tokens, DIM=1024, E=8 experts, INTER=1408, top-2 routing.
Reference computes all experts densely then gathers; we instead route on the
host (sort token-slots by expert), assign one expert per core, and each core
runs a SwiGLU FFN over only its routed tokens (padded to a common capacity so
all 8 cores execute the same SPMD program).

Tokens whose two routed experts coincide are computed once and scattered to
both output slots (~6% of slots for iid top-2 routing).

Precision: w1/w3 are stored as fp8 e3m4 scaled by 64 (weights are iid
N(0, 1/1024), so x64 centers them in e3m4's normal range; measured end-to-end
rel err ~1.5e-2 vs the 2e-2 gate). The matmul runs mixed fp8e3 x bf16 at full
PE rate, PSUM holds 64*h; the silu activation applies scale=1/64 and the gate
product keeps the 64x on h3, which is cancelled by pre-dividing w2 by 64 on
the host (exact power-of-2). This halves the w1/w3 HBM bytes that gate the
DMA-ramp-bound head of the kernel.

Device layout (per core, tokens on the free dim):
  xt[j]   [P,KT,n_j] bf16  x_gathered.T, one contiguous block per chunk
  w1g/w3g [P,G,KT*P] fp8e3 w.T m-tile groups (m0 | m1 | m2-3 | m4-6 | m7-10),
                           partition-major so each group is one cheap DMA
                           trigger (~600ns of descriptor writes) delivered in
                           consumption order
  w2t     [P,MT,DIM] bf16  w2.T/64, partition-major, single trigger
  yt      [KT,P,C]   bf16  y.T (host upcasts to f32)

Compute per core (PE-serial; phase B of chunk j runs before phase A of j+1):
  A: per m: 64*h1.T = w1[m] @ x.T (8 k-tile matmuls into PSUM), same for h3;
     g.T = silu(h1.T) * 64*h3.T -> SBUF bf16
  B: i-pair passes with m outer, so w2 m-tiles are consumed progressively as
     they stream in; the very last output tile is split column-wise and
     drained via parallel vector/scalar casts + sync/tensor-queue DMAs to
     shorten the post-final-matmul chain.
"""

import numpy as np
import ml_dtypes

T, DIM, E, INTER, TOPK = 4096, 1024, 8, 1408, 2
NCORES = 8
P = 128
KT = DIM // P    # 8 k-tiles over DIM
MT = INTER // P  # 11 m-tiles over INTER
WGROUPS = [(0, 1), (1, 2), (2, 4), (4, 7), (7, 11)]  # m-tile delivery groups
TAIL = 232       # final drain piece: 11 FD=232 matmuls == 11 LDWEIGHTS, balanced

TRACE = False  # test.py sets this to capture an NTFF profile
LAST_RESULTS = None  # BassKernelResults of the last run (for test.py)

_NC_CACHE = {}


def _chunks_for(C):
    # Split C into chunks of at most 512 (PSUM bank = 512 fp32). Fewer
    # chunks = fewer matmuls (each costs ~3ns of issue overhead), so two
    # chunks for C<=1024, the smaller first (its m0 phase rides the DMA
    # ramp).
    if C <= 512:
        return [C]
    if C <= 1024:
        # two chunks, smaller first (its m0 phase rides the DMA ramp)
        n0 = max(128, C - 512)
        n1 = C - n0
        assert 0 < n0 <= 512 and 0 < n1 <= 512, (C, n0, n1)
        return [n0, n1]
    # fallback: equal-ish chunks of <=512
    nch = -(-C // 512)
    base = C // nch
    out = []
    rem = C
    for i in range(nch, 0, -1):
        n = min(512, -(-rem // i))
        n = -(-n // 8) * 8 if i > 1 else rem
        n = min(n, 512, rem)
        out.append(n)
        rem -= n
    assert sum(out) == C and all(0 < n <= 512 for n in out), out
    return out


def _build_nc(C):
    import concourse.mybir as mybir
    import concourse.tile as tile
    from concourse import bacc

    dt = mybir.dt
    AF = mybir.ActivationFunctionType
    chunks = _chunks_for(C)
    nchunks = len(chunks)

    nc = bacc.Bacc(
        "TRN2", target_bir_lowering=False, debug=False, enable_asserts=False
    )
    xts = [
        nc.dram_tensor(f"xt{j}", [P, KT, n], dt.bfloat16, kind="ExternalInput")
        for j, n in enumerate(chunks)
    ]
    w1gs = [
        nc.dram_tensor(f"w1g{i}", [P, b - a, KT * P], dt.float8e3, kind="ExternalInput")
        for i, (a, b) in enumerate(WGROUPS)
    ]
    w3gs = [
        nc.dram_tensor(f"w3g{i}", [P, b - a, KT * P], dt.float8e3, kind="ExternalInput")
        for i, (a, b) in enumerate(WGROUPS)
    ]
    w2t = nc.dram_tensor("w2t", [P, MT, DIM], dt.bfloat16, kind="ExternalInput")
    yt = nc.dram_tensor("yt", [KT, P, C], dt.bfloat16, kind="ExternalOutput")

    with tile.TileContext(nc) as tc:
        with (
            tc.tile_pool(name="persist", bufs=1) as wpool,
            tc.tile_pool(name="gbuf", bufs=3) as gpool,
            tc.tile_pool(name="ybuf", bufs=4) as ypool,
            tc.tile_pool(name="silbuf", bufs=3) as spool,
            tc.tile_pool(name="psA", bufs=2, space="PSUM") as psA,
            tc.tile_pool(name="psB", bufs=3, space="PSUM") as psB,
        ):
            xss = [wpool.tile([P, KT, n], dt.bfloat16, name=f"xs{j}")
                   for j, n in enumerate(chunks)]
            w1s = wpool.tile([P, MT, KT * P], dt.float8e3)
            w3s = wpool.tile([P, MT, KT * P], dt.float8e3)
            w2s = wpool.tile([P, MT, DIM], dt.bfloat16)

            # (Tried and rejected: pre-warming the HAM clock gate with dummy
            # matmuls before the first DMA lands. The dummies delay the
            # first real matmul past its data-ready time, and m0's DMA gaps
            # re-cool the clock anyway -- measured net loss ~2us.)

            # All input DMAs on the sync queue in consumption-deadline order.
            # Descriptor writes cost ~650ns of serialized sequencer time per
            # dma_start, so the trigger count is the head's second constraint
            # besides ramp bandwidth (~215 GB/s): the critical prefix uses a
            # few fine triggers (half-m0 w1, k-pair x blocks), everything
            # else is one contiguous trigger per weight m-group.
            H = KT * P // 2
            x0 = xss[0]
            nc.sync.dma_start(w1s[:, 0, :H], w1gs[0][:, 0, :H])
            nc.sync.dma_start(x0[:, 0:2, :], xts[0][:, 0:2, :])
            nc.sync.dma_start(w1s[:, 0, H:], w1gs[0][:, 0, H:])
            nc.sync.dma_start(w3s[:, 0, :H], w3gs[0][:, 0, :H])
            nc.sync.dma_start(x0[:, 2:4, :], xts[0][:, 2:4, :])
            nc.sync.dma_start(w3s[:, 0, H:], w3gs[0][:, 0, H:])
            nc.sync.dma_start(x0[:, 4:6, :], xts[0][:, 4:6, :])
            nc.sync.dma_start(x0[:, 6:8, :], xts[0][:, 6:8, :])
            for i, (a, b) in list(enumerate(WGROUPS))[1:]:
                nc.sync.dma_start(w1s[:, a:b, :], w1gs[i][:])
                nc.sync.dma_start(w3s[:, a:b, :], w3gs[i][:])
            nc.sync.dma_start(w2s[:], w2t[:])
            for j in range(1, nchunks):
                nc.sync.dma_start(xss[j][:], xts[j][:])

            c0 = 0
            for j, n in enumerate(chunks):
                xsj = xss[j]
                last_chunk = j == nchunks - 1
                gs = gpool.tile([P, MT, n], dt.bfloat16, name="gs")

                # ---- phase A: 64*h1 / 64*h3, silu-gate into gs ----
                for m in range(MT):
                    p1 = psA.tile([P, n], dt.float32, name="p1")
                    p3 = psA.tile([P, n], dt.float32, name="p3")
                    if j == 0 and m == 0:
                        # Emit in DMA-arrival order (w1 half, x k-pairs, w3
                        # half, ...) so the PE computes through the ramp.
                        seq = []
                        for kk in range(0, KT, 2):
                            seq += [(p1, w1s, kk), (p1, w1s, kk + 1),
                                    (p3, w3s, kk), (p3, w3s, kk + 1)]
                    else:
                        seq = [(p1, w1s, k) for k in range(KT)] + \
                              [(p3, w3s, k) for k in range(KT)]
                    seen = {}
                    for dst, wsrc, k in seq:
                        first = id(dst) not in seen
                        seen[id(dst)] = seen.get(id(dst), 0) + 1
                        nc.tensor.matmul(
                            dst[:],
                            wsrc[:, m, k * P:(k + 1) * P],
                            xsj[:, k, :],
                            start=first,
                            stop=(seen[id(dst)] == KT),
                        )
                    sil = spool.tile([P, n], dt.bfloat16, name="sil")
                    nc.scalar.activation(sil[:], p1[:], AF.Silu, scale=1.0 / 64.0)
                    nc.vector.tensor_mul(gs[:, m, :], sil[:], p3[:])

                # ---- phase B: i-pair passes, m outer (w2 streams in m
                # order, so pairing i keeps the w2 deadline progressive).
                # For the last chunk, the big part of output tile 7 is
                # computed in the FIRST pass so that only a TAIL-column
                # piece remains to drain after the very last matmul; the
                # (5,6) pass splits its casts/DMAs across scalar/vector and
                # scalar/sync so nothing congests the final drain chains.
                split_last = last_chunk and n - TAIL >= 128
                nb = n - TAIL
                if split_last:
                    # Output tile 7's big part pairs with a full tile
                    # mid-phase; tiles 5 and 6 run as singleton passes with
                    # their DMAs on the sync queue, so the scalar queue is
                    # idle (and the vector engine nearly so) when the TAIL
                    # piece drains after the very last matmul.
                    passes = [((0, n), (1, n)), ((2, n), (3, n)),
                              ((4, n), (7, nb)), ((5, n),), ((6, n),),
                              ("tail",)]
                else:
                    passes = [((0, n), (1, n)), ((2, n), (3, n)),
                              ((4, n), (5, n)), ((6, n), (7, n))]
                qtoggle = 0
                for pp in passes:
                    if pp != ("tail",):
                        pys = [psB.tile([P, w], dt.float32, name="py")
                               for (_, w) in pp]
                        for m in range(MT):
                            for pi, (i, w) in enumerate(pp):
                                nc.tensor.matmul(
                                    pys[pi][:],
                                    w2s[:, m, i * P:(i + 1) * P],
                                    gs[:, m, :w],
                                    start=(m == 0),
                                    stop=(m == MT - 1),
                                )
                        singleton = len(pp) == 1
                        for pi, (i, w) in enumerate(pp):
                            ys = ypool.tile([P, w], dt.bfloat16, name="ys")
                            nc.vector.tensor_copy(ys[:], pys[pi][:])
                            if singleton:
                                # i=5 -> sync (2.3us before the end, clears
                                # easily); i=6 -> scalar, so its 131KB
                                # transfer doesn't queue ahead of the tail's
                                # sync-queue half.
                                q = nc.sync if i == 5 else nc.scalar
                            else:
                                q = nc.sync if qtoggle % 2 == 0 else nc.scalar
                                qtoggle += 1
                            q.dma_start(yt[i, :, c0:c0 + w], ys[:])
                    else:
                        # TAIL piece of output tile 7: the only work left
                        # after pass (5,6); drains via two parallel
                        # cast+DMA chains (vector->sync, scalar->scalar).
                        pyb = psB.tile([P, TAIL], dt.float32, name="py")
                        for m in range(MT):
                            nc.tensor.matmul(
                                pyb[:], w2s[:, m, 7 * P:8 * P], gs[:, m, nb:],
                                start=(m == 0), stop=(m == MT - 1),
                            )
                        # Both casts on vector (the scalar engine has a
                        # ~0.45us semaphore pickup lag); DMAs split across
                        # the sync and (idle) scalar queues.
                        h = TAIL // 2
                        yb1 = ypool.tile([P, h], dt.bfloat16, name="yb1")
                        yb2 = ypool.tile([P, TAIL - h], dt.bfloat16, name="yb2")
                        nc.vector.tensor_copy(yb1[:], pyb[:, :h])
                        nc.vector.tensor_copy(yb2[:], pyb[:, h:])
                        nc.sync.dma_start(yt[7, :, c0 + nb:c0 + nb + h], yb1[:])
                        nc.scalar.dma_start(yt[7, :, c0 + nb + h:c0 + n], yb2[:])
                c0 += n

    nc.compile()
    return nc


def _get_nc(C):
    if C not in _NC_CACHE:
        _NC_CACHE[C] = _build_nc(C)
    return _NC_CACHE[C]


def _ensure_ntff_hook_importable():
    # bass_utils imports antenv.axon_hooks when tracing is requested; in
    # containers whose antenv stub lacks that submodule the import would
    # crash. Register a null hook so tracing degrades to "no trace".
    import sys
    import types

    try:
        import antenv.axon_hooks  # noqa: F401
    except ImportError:
        mod = types.ModuleType("antenv.axon_hooks")
        mod.get_axon_ntff_profile_hook = lambda: None
        mod.set_axon_ntff_profile_hook = lambda hook: None
        sys.modules["antenv.axon_hooks"] = mod


def kernel(x, expert_indices, w1, w2, w3):
    global LAST_RESULTS
    import os
    import sys

    # The bass kernel executes on the NeuronCores via the axon PJRT backend;
    # a JAX_PLATFORMS=cpu pin would hide those devices. Clear it if jax
    # hasn't initialized yet.
    if os.environ.get("JAX_PLATFORMS") == "cpu" and "jax" not in sys.modules:
        del os.environ["JAX_PLATFORMS"]

    from concourse import bass_utils

    _ensure_ntff_hook_importable()
    x = np.asarray(x, dtype=np.float32)
    idx = np.asarray(expert_indices)
    w1 = np.asarray(w1, dtype=np.float32)
    w2 = np.asarray(w2, dtype=np.float32)
    w3 = np.asarray(w3, dtype=np.float32)

    bf16 = ml_dtypes.bfloat16
    e3m4 = ml_dtypes.float8_e3m4

    # --- host routing: stable-sort the (token, k) slots by expert id,
    # dropping slots whose (token, expert) pair duplicates slot k=0 ---
    flat = idx.reshape(-1).astype(np.int64)  # slot s = t*TOPK + k -> expert
    keep = np.ones(T * TOPK, dtype=bool)
    dup = idx[:, 1] == idx[:, 0]
    keep[1::2] = ~dup
    kept_slots = np.nonzero(keep)[0]
    kept_flat = flat[keep]
    order = np.argsort(kept_flat, kind="stable")  # kept slots grouped by expert
    sorted_slots = kept_slots[order]
    counts = np.bincount(kept_flat, minlength=E)
    starts = np.zeros(E + 1, dtype=np.int64)
    np.cumsum(counts, out=starts[1:])
    cmax = int(counts.max())
    C = max(256, -(-cmax // 8) * 8)  # pad capacity to a multiple of 8

    nc = _get_nc(C)

    chunks = _chunks_for(C)
    bounds = np.cumsum([0] + chunks)
    xb = x.astype(bf16)
    in_maps = []
    for e in range(E):
        slots = sorted_slots[starts[e]:starts[e + 1]]
        tokens = slots // TOPK
        xg = np.zeros((C, DIM), dtype=bf16)
        xg[: len(tokens)] = xb[tokens]
        # [C, DIM] -> [P, KT, C] (partition-major), then per-chunk blocks
        xpkc = xg.T.reshape(KT, P, C).transpose(1, 0, 2)
        im = {
            f"xt{j}": np.ascontiguousarray(xpkc[:, :, bounds[j]:bounds[j + 1]])
            for j in range(len(chunks))
        }
        # wt[m, p, k*128+j] = w[e][m*128+j, k*128+p]; fp8 e3m4 scaled x64,
        # then regrouped partition-major per m-group for contiguous DMAs.
        for name, w in (("w1", w1), ("w3", w3)):
            wq = (w[e] * 64.0).astype(e3m4)
            wt = wq.reshape(MT, P, KT, P).transpose(0, 3, 2, 1).reshape(MT, P, KT * P)
            for i, (a, b) in enumerate(WGROUPS):
                im[f"{name}g{i}"] = np.ascontiguousarray(wt[a:b].transpose(1, 0, 2))
        # w2t[p, m, i] = w2[e][i, m*128+p] / 64 (cancels the x64 on h3)
        im["w2t"] = np.ascontiguousarray(
            (w2[e].T / 64.0).astype(bf16).reshape(MT, P, DIM).transpose(1, 0, 2)
        )
        in_maps.append(im)

    res = bass_utils.run_bass_kernel_spmd(
        nc, in_maps, core_ids=list(range(NCORES)), trace=TRACE
    )
    LAST_RESULTS = res

    out = np.empty((T * TOPK, DIM), dtype=np.float32)
    for e in range(E):
        slots = sorted_slots[starts[e]:starts[e + 1]]
        yte = res.results[e]["yt"]  # [KT, P, C] bf16
        y = yte.reshape(DIM, C).astype(np.float32)  # y.T
        out[slots] = y[:, : len(slots)].T
    out = out.reshape(T, TOPK, DIM)
    out[dup, 1] = out[dup, 0]  # slots dropped by dedupe share the k=0 result
    return out


# revision 26
# speedup vs baseline: 1.0029x; 1.0029x over previous
"""Expert-parallel MoE SwiGLU FFN kernel for 8 Trainium2 NeuronCores.

Problem: T=4096 tokens, DIM=1024, E=8 experts, INTER=1408, top-2 routing.
Reference computes all experts densely then gathers; we instead route on the
host (sort token-slots by expert), assign one expert per core, and each core
runs a SwiGLU FFN over only its routed tokens (padded to a common capacity so
all 8 cores execute the same SPMD program).

Tokens whose two routed experts coincide are computed once and scattered to
both output slots (~6% of slots for iid top-2 routing).

Precision: w1/w3 are stored as fp8 e3m4 scaled by 64 (weights are iid
N(0, 1/1024), so x64 centers them in e3m4's normal range; measured end-to-end
rel err ~1.5e-2 vs the 2e-2 gate). The matmul runs mixed fp8e3 x bf16 at full
PE rate, PSUM holds 64*h; the silu activation applies scale=1/64 and the gate
product keeps the 64x on h3, which is cancelled by pre-dividing w2 by 64 on
the host (exact power-of-2). This halves the w1/w3 HBM bytes that gate the
DMA-ramp-bound head of the kernel.

Device layout (per core, tokens on the free dim):
  xt[j]   [P,KT,n_j] bf16  x_gathered.T, one contiguous block per chunk
  w1g/w3g [P,G,KT*P] fp8e3 w.T m-tile groups (m0 | m1 | m2-3 | m4-6 | m7-10),
                           partition-major so each group is one cheap DMA
                           trigger (~600ns of descriptor writes) delivered in
                           consumption order
  w2t     [P,MT,DIM] bf16  w2.T/64, partition-major, single trigger
  yt      [KT,P,C]   bf16  y.T (host upcasts to f32)

Compute per core (PE-serial; phase B of chunk j runs before phase A of j+1):
  A: per m: 64*h1.T = w1[m] @ x.T (8 k-tile matmuls into PSUM), same for h3;
     g.T = silu(h1.T) * 64*h3.T -> SBUF bf16
  B: i-pair passes with m outer, so w2 m-tiles are consumed progressively as
     they stream in; the very last output tile is split column-wise and
     drained via parallel vector/scalar casts + sync/tensor-queue DMAs to
     shorten the post-final-matmul chain.
"""

import numpy as np
import ml_dtypes

T, DIM, E, INTER, TOPK = 4096, 1024, 8, 1408, 2
NCORES = 8
P = 128
KT = DIM // P    # 8 k-tiles over DIM
MT = INTER // P  # 11 m-tiles over INTER
WGROUPS = [(0, 1), (1, 2), (2, 4), (4, 7), (7, 11)]  # m-tile delivery groups
TAIL = 232       # final drain piece: 11 FD=232 matmuls == 11 LDWEIGHTS, balanced

TRACE = False  # test.py sets this to capture an NTFF profile
LAST_RESULTS = None  # BassKernelResults of the last run (for test.py)

_NC_CACHE = {}


def _chunks_for(C):
    # Split C into chunks of at most 512 (PSUM bank = 512 fp32). Fewer
    # chunks = fewer matmuls (each costs ~3ns of issue overhead), so two
    # chunks for C<=1024, the smaller first (its m0 phase rides the DMA
    # ramp).
    if C <= 512:
        return [C]
    if C <= 1024:
        # two chunks, smaller first (its m0 phase rides the DMA ramp)
        n0 = max(128, C - 512)
        n1 = C - n0
        assert 0 < n0 <= 512 and 0 < n1 <= 512, (C, n0, n1)
        return [n0, n1]
    # fallback: equal-ish chunks of <=512
    nch = -(-C // 512)
    base = C // nch
    out = []
    rem = C
    for i in range(nch, 0, -1):
        n = min(512, -(-rem // i))
        n = -(-n // 8) * 8 if i > 1 else rem
        n = min(n, 512, rem)
        out.append(n)
        rem -= n
    assert sum(out) == C and all(0 < n <= 512 for n in out), out
    return out


def _build_nc(C):
    import concourse.mybir as mybir
    import concourse.tile as tile
    from concourse import bacc

    dt = mybir.dt
    AF = mybir.ActivationFunctionType
    chunks = _chunks_for(C)
    nchunks = len(chunks)

    nc = bacc.Bacc(
        "TRN2", target_bir_lowering=False, debug=False, enable_asserts=False
    )
    xts = [
        nc.dram_tensor(f"xt{j}", [P, KT, n], dt.bfloat16, kind="ExternalInput")
        for j, n in enumerate(chunks)
    ]
    w1gs = [
        nc.dram_tensor(f"w1g{i}", [P, b - a, KT * P], dt.float8e3, kind="ExternalInput")
        for i, (a, b) in enumerate(WGROUPS)
    ]
    w3gs = [
        nc.dram_tensor(f"w3g{i}", [P, b - a, KT * P], dt.float8e3, kind="ExternalInput")
        for i, (a, b) in enumerate(WGROUPS)
    ]
    w2t = nc.dram_tensor("w2t", [P, MT, DIM], dt.bfloat16, kind="ExternalInput")
    yt = nc.dram_tensor("yt", [KT, P, C], dt.bfloat16, kind="ExternalOutput")

    with tile.TileContext(nc) as tc:
        with (
            tc.tile_pool(name="persist", bufs=1) as wpool,
            tc.tile_pool(name="gbuf", bufs=3) as gpool,
            tc.tile_pool(name="ybuf", bufs=4) as ypool,
            tc.tile_pool(name="silbuf", bufs=3) as spool,
            tc.tile_pool(name="psA", bufs=2, space="PSUM") as psA,
            tc.tile_pool(name="psB", bufs=3, space="PSUM") as psB,
        ):
            xss = [wpool.tile([P, KT, n], dt.bfloat16, name=f"xs{j}")
                   for j, n in enumerate(chunks)]
            w1s = wpool.tile([P, MT, KT * P], dt.float8e3)
            w3s = wpool.tile([P, MT, KT * P], dt.float8e3)
            w2s = wpool.tile([P, MT, DIM], dt.bfloat16)

            # (Tried and rejected: pre-warming the HAM clock gate with dummy
            # matmuls before the first DMA lands. The dummies delay the
            # first real matmul past its data-ready time, and m0's DMA gaps
            # re-cool the clock anyway -- measured net loss ~2us.)

            # All input DMAs on the sync queue in consumption-deadline order.
            # Descriptor writes cost ~650ns of serialized sequencer time per
            # dma_start, so the trigger count is the head's second constraint
            # besides ramp bandwidth (~215 GB/s): the critical prefix uses a
            # few fine triggers (half-m0 w1, k-pair x blocks), everything
            # else is one contiguous trigger per weight m-group.
            H = KT * P // 2
            x0 = xss[0]
            nc.sync.dma_start(w1s[:, 0, :H], w1gs[0][:, 0, :H])
            nc.sync.dma_start(x0[:, 0:2, :], xts[0][:, 0:2, :])
            nc.sync.dma_start(w3s[:, 0, :H], w3gs[0][:, 0, :H])
            nc.sync.dma_start(x0[:, 2:4, :], xts[0][:, 2:4, :])
            nc.sync.dma_start(w1s[:, 0, H:], w1gs[0][:, 0, H:])
            nc.sync.dma_start(w3s[:, 0, H:], w3gs[0][:, 0, H:])
            nc.sync.dma_start(x0[:, 4:6, :], xts[0][:, 4:6, :])
            nc.sync.dma_start(x0[:, 6:8, :], xts[0][:, 6:8, :])
            for i, (a, b) in list(enumerate(WGROUPS))[1:]:
                nc.sync.dma_start(w1s[:, a:b, :], w1gs[i][:])
                nc.sync.dma_start(w3s[:, a:b, :], w3gs[i][:])
            nc.sync.dma_start(w2s[:], w2t[:])
            for j in range(1, nchunks):
                nc.sync.dma_start(xss[j][:], xts[j][:])

            c0 = 0
            for j, n in enumerate(chunks):
                xsj = xss[j]
                last_chunk = j == nchunks - 1
                gs = gpool.tile([P, MT, n], dt.bfloat16, name="gs")

                # ---- phase A: 64*h1 / 64*h3, silu-gate into gs ----
                for m in range(MT):
                    p1 = psA.tile([P, n], dt.float32, name="p1")
                    p3 = psA.tile([P, n], dt.float32, name="p3")
                    if j == 0 and m == 0:
                        # Emit in DMA-arrival order (w1 half, x k-pairs, w3
                        # half, ...) so the PE computes through the ramp.
                        seq = []
                        for kk in range(0, KT, 2):
                            seq += [(p1, w1s, kk), (p1, w1s, kk + 1),
                                    (p3, w3s, kk), (p3, w3s, kk + 1)]
                    else:
                        seq = [(p1, w1s, k) for k in range(KT)] + \
                              [(p3, w3s, k) for k in range(KT)]
                    seen = {}
                    for dst, wsrc, k in seq:
                        first = id(dst) not in seen
                        seen[id(dst)] = seen.get(id(dst), 0) + 1
                        nc.tensor.matmul(
                            dst[:],
                            wsrc[:, m, k * P:(k + 1) * P],
                            xsj[:, k, :],
                            start=first,
                            stop=(seen[id(dst)] == KT),
                        )
                    sil = spool.tile([P, n], dt.bfloat16, name="sil")
                    nc.scalar.activation(sil[:], p1[:], AF.Silu, scale=1.0 / 64.0)
                    nc.vector.tensor_mul(gs[:, m, :], sil[:], p3[:])

                # ---- phase B: i-pair passes, m outer (w2 streams in m
                # order, so pairing i keeps the w2 deadline progressive).
                # For the last chunk, the big part of output tile 7 is
                # computed in the FIRST pass so that only a TAIL-column
                # piece remains to drain after the very last matmul; the
                # (5,6) pass splits its casts/DMAs across scalar/vector and
                # scalar/sync so nothing congests the final drain chains.
                split_last = last_chunk and n - TAIL >= 128
                nb = n - TAIL
                if split_last:
                    # Output tile 7's big part pairs with a full tile
                    # mid-phase; tiles 5 and 6 run as singleton passes with
                    # their DMAs on the sync queue, so the scalar queue is
                    # idle (and the vector engine nearly so) when the TAIL
                    # piece drains after the very last matmul.
                    passes = [((0, n), (1, n)), ((2, n), (3, n)),
                              ((4, n), (7, nb)), ((5, n),), ((6, n),),
                              ("tail",)]
                else:
                    passes = [((0, n), (1, n)), ((2, n), (3, n)),
                              ((4, n), (5, n)), ((6, n), (7, n))]
                qtoggle = 0
                for pp in passes:
                    if pp != ("tail",):
                        pys = [psB.tile([P, w], dt.float32, name="py")
                               for (_, w) in pp]
                        for m in range(MT):
                            for pi, (i, w) in enumerate(pp):
                                nc.tensor.matmul(
                                    pys[pi][:],
                                    w2s[:, m, i * P:(i + 1) * P],
                                    gs[:, m, :w],
                                    start=(m == 0),
                                    stop=(m == MT - 1),
                                )
                        singleton = len(pp) == 1
                        for pi, (i, w) in enumerate(pp):
                            ys = ypool.tile([P, w], dt.bfloat16, name="ys")
                            nc.vector.tensor_copy(ys[:], pys[pi][:])
                            if singleton:
                                # i=5 -> sync (2.3us before the end, clears
                                # easily); i=6 -> scalar, so its 131KB
                                # transfer doesn't queue ahead of the tail's
                                # sync-queue half.
                                q = nc.sync if i == 5 else nc.scalar
                            else:
                                q = nc.sync if qtoggle % 2 == 0 else nc.scalar
                                qtoggle += 1
                            q.dma_start(yt[i, :, c0:c0 + w], ys[:])
                    else:
                        # TAIL piece of output tile 7: the only work left
                        # after pass (5,6); drains via two parallel
                        # cast+DMA chains (vector->sync, scalar->scalar).
                        pyb = psB.tile([P, TAIL], dt.float32, name="py")
                        for m in range(MT):
                            nc.tensor.matmul(
                                pyb[:], w2s[:, m, 7 * P:8 * P], gs[:, m, nb:],
                                start=(m == 0), stop=(m == MT - 1),
                            )
                        # Single unsplit piece: yt's column slice is one
                        # contiguous 464B run per partition, so one DMA of
                        # 128 descriptors; splitting in half would double
                        # the descriptor count (the engines pay ~42ns per
                        # descriptor regardless of size) and serialize two
                        # casts on the vector engine.
                        yb = ypool.tile([P, TAIL], dt.bfloat16, name="yb")
                        nc.vector.tensor_copy(yb[:], pyb[:])
                        nc.sync.dma_start(yt[7, :, c0 + nb:c0 + n], yb[:])
                c0 += n

    nc.compile()
    return nc


def _get_nc(C):
    if C not in _NC_CACHE:
        _NC_CACHE[C] = _build_nc(C)
    return _NC_CACHE[C]


def _ensure_ntff_hook_importable():
    # bass_utils imports antenv.axon_hooks when tracing is requested; in
    # containers whose antenv stub lacks that submodule the import would
    # crash. Register a null hook so tracing degrades to "no trace".
    import sys
    import types

    try:
        import antenv.axon_hooks  # noqa: F401
    except ImportError:
        mod = types.ModuleType("antenv.axon_hooks")
        mod.get_axon_ntff_profile_hook = lambda: None
        mod.set_axon_ntff_profile_hook = lambda hook: None
        sys.modules["antenv.axon_hooks"] = mod


def kernel(x, expert_indices, w1, w2, w3):
    global LAST_RESULTS
    import os
    import sys

    # The bass kernel executes on the NeuronCores via the axon PJRT backend;
    # a JAX_PLATFORMS=cpu pin would hide those devices. Clear it if jax
    # hasn't initialized yet.
    if os.environ.get("JAX_PLATFORMS") == "cpu" and "jax" not in sys.modules:
        del os.environ["JAX_PLATFORMS"]

    from concourse import bass_utils

    _ensure_ntff_hook_importable()
    x = np.asarray(x, dtype=np.float32)
    idx = np.asarray(expert_indices)
    w1 = np.asarray(w1, dtype=np.float32)
    w2 = np.asarray(w2, dtype=np.float32)
    w3 = np.asarray(w3, dtype=np.float32)

    bf16 = ml_dtypes.bfloat16
    e3m4 = ml_dtypes.float8_e3m4

    # --- host routing: stable-sort the (token, k) slots by expert id,
    # dropping slots whose (token, expert) pair duplicates slot k=0 ---
    flat = idx.reshape(-1).astype(np.int64)  # slot s = t*TOPK + k -> expert
    keep = np.ones(T * TOPK, dtype=bool)
    dup = idx[:, 1] == idx[:, 0]
    keep[1::2] = ~dup
    kept_slots = np.nonzero(keep)[0]
    kept_flat = flat[keep]
    order = np.argsort(kept_flat, kind="stable")  # kept slots grouped by expert
    sorted_slots = kept_slots[order]
    counts = np.bincount(kept_flat, minlength=E)
    starts = np.zeros(E + 1, dtype=np.int64)
    np.cumsum(counts, out=starts[1:])
    cmax = int(counts.max())
    C = max(256, -(-cmax // 8) * 8)  # pad capacity to a multiple of 8

    nc = _get_nc(C)

    chunks = _chunks_for(C)
    bounds = np.cumsum([0] + chunks)
    xb = x.astype(bf16)
    in_maps = []
    for e in range(E):
        slots = sorted_slots[starts[e]:starts[e + 1]]
        tokens = slots // TOPK
        xg = np.zeros((C, DIM), dtype=bf16)
        xg[: len(tokens)] = xb[tokens]
        # [C, DIM] -> [P, KT, C] (partition-major), then per-chunk blocks
        xpkc = xg.T.reshape(KT, P, C).transpose(1, 0, 2)
        im = {
            f"xt{j}": np.ascontiguousarray(xpkc[:, :, bounds[j]:bounds[j + 1]])
            for j in range(len(chunks))
        }
        # wt[m, p, k*128+j] = w[e][m*128+j, k*128+p]; fp8 e3m4 scaled x64,
        # then regrouped partition-major per m-group for contiguous DMAs.
        for name, w in (("w1", w1), ("w3", w3)):
            wq = (w[e] * 64.0).astype(e3m4)
            wt = wq.reshape(MT, P, KT, P).transpose(0, 3, 2, 1).reshape(MT, P, KT * P)
            for i, (a, b) in enumerate(WGROUPS):
                im[f"{name}g{i}"] = np.ascontiguousarray(wt[a:b].transpose(1, 0, 2))
        # w2t[p, m, i] = w2[e][i, m*128+p] / 64 (cancels the x64 on h3)
        im["w2t"] = np.ascontiguousarray(
            (w2[e].T / 64.0).astype(bf16).reshape(MT, P, DIM).transpose(1, 0, 2)
        )
        in_maps.append(im)

    res = bass_utils.run_bass_kernel_spmd(
        nc, in_maps, core_ids=list(range(NCORES)), trace=TRACE
    )
    LAST_RESULTS = res

    out = np.empty((T * TOPK, DIM), dtype=np.float32)
    for e in range(E):
        slots = sorted_slots[starts[e]:starts[e + 1]]
        yte = res.results[e]["yt"]  # [KT, P, C] bf16
        y = yte.reshape(DIM, C).astype(np.float32)  # y.T
        out[slots] = y[:, : len(slots)].T
    out = out.reshape(T, TOPK, DIM)
    out[dup, 1] = out[dup, 0]  # slots dropped by dedupe share the k=0 result
    return out
